# revision 15
# baseline (speedup 1.0000x reference)
"""Radial power-spectrum (GroupStat.get_spectrum) Trainium2 kernel.

Math:  out[b,c,r] = sum_{p: idx[p]==r} x[b,c,p]^2 * w[p] / (cnt[r]+eps)

Strategy (8 NeuronCores, sharded over PIXELS, not batch):
  * All B*C = 1024 (b,c) rows on every core; each core owns 4096 of the
    33024 pixels (cores cover 32768; the 256-pixel remainder is summed
    exactly on the host in fp32 and added to the gathered result).
  * Host prep: transpose x to pixel-major [NPIX, 1024], scale by 32 and
    cast to fp16.  With 1024 rows per pixel the DMA lines are 2 KB, so
    the load runs at full HBM bandwidth AND lands with pixel on the
    partition dim -- no on-device transpose at all.
  * Device pipeline per 128-pixel chunk (32 per core):
      - DMA fp16 tiles [128p, 4, 1024n] (4 chunks per DMA; the last two
        tiles are 2 chunks so the post-DMA tail is short)
      - square in fp16 (values are 1024*x^2; the 32x host prescale keeps
        tiny x^2 out of fp16 subnormals), per-chunk ops balanced between
        ScalarE (14 chunks @1038ns) and DVE (18 chunks @594ns)
      - DVE: weighted one-hot [128p, 130r] = (iota == idx[p]) * wt[p],
        built ONCE per chunk and reused by all 8 row-groups
      - PE: for each of 8 row-groups g: psum_g[128n,130r] += x2T_g @ oh
  * psum_g -> SBUF f32 (4 copies on ScalarE, 4 on DVE, concurrently),
    two output DMAs overlap the copies; host sums the 8 per-core
    partials (pixel sharding => partial shell sums) and divides by 1024.
"""

import numpy as np

from concourse import bass, bacc, mybir
import concourse.tile as tile
from concourse.bass_utils import run_bass_kernel_spmd

B, C, S, XDIM = 128, 8, 256, 129
MAX_R = XDIM                # 129 shells
EPS = 1e-5
NCORES = 8
NROW = B * C                # 1024 total (b,c) rows
NGRP = NROW // 128          # 8 row-groups of 128
NPIX = S * XDIM             # 33024 pixels
NCH = 32                    # chunks of 128 pixels per core
CPIX = NCH * 128            # 4096 pixels per core
NPIX_DEV = NCORES * CPIX    # 32768 on device; 256 residual on host
RPAD = 130                  # even free dim for DVE 4x mode; col 129 unused
TILES = [4] * 7 + [2, 2]    # chunks per DMA tile (sum = 32); tapered tail
PRESCALE = 32.0             # host multiplies x by 32 -> squares are 1024*x^2
# chunks whose square runs on ScalarE (~14/32; DVE also builds one-hots)
ACTE_CHUNKS = {0, 2, 4, 6, 9, 11, 13, 16, 18, 20, 22, 25, 27, 30}

F32 = mybir.dt.float32
F16 = mybir.dt.float16

_CACHE: dict = {}


def _build_program():
    nc = bacc.Bacc("TRN2", target_bir_lowering=False, debug=False,
                   num_devices=NCORES)

    # x, pre-transposed+scaled+fp16 on host: [chunk, pixel-in-chunk, row]
    x_d = nc.dram_tensor("xt", [NCH, 128, NROW], F16,
                         kind="ExternalInput").ap()
    # idx and wt packed: col c = idx for chunk c, col NCH+c = wt for chunk c
    iw_d = nc.dram_tensor("iw", [128, 2 * NCH], F32,
                          kind="ExternalInput").ap()
    iota_d = nc.dram_tensor("iota", [128, RPAD], F16,
                            kind="ExternalInput").ap()
    out_d = nc.dram_tensor("out", [128, NGRP * MAX_R], F32,
                           kind="ExternalOutput").ap()

    with tile.TileContext(nc) as tc:
        with tc.tile_pool(name="const", bufs=1) as const_pool, \
             tc.tile_pool(name="xin", bufs=4) as xin_pool, \
             tc.tile_pool(name="x2", bufs=4) as x2_pool, \
             tc.tile_pool(name="oh", bufs=8) as oh_pool, \
             tc.tile_pool(name="acc", bufs=1, space="PSUM") as acc_pool:

            accs = [acc_pool.tile([128, RPAD], F32, name=f"acc{g}")
                    for g in range(NGRP)]
            iw_t = const_pool.tile([128, 2 * NCH], F32)
            iota_t = const_pool.tile([128, RPAD], F16)

            c0 = 0
            first = True
            for tch in TILES:
                xin = xin_pool.tile([128, 4, NROW], F16, tag="xin")
                nc.sync.dma_start(
                    xin[:, :tch], x_d[c0:c0 + tch].rearrange("c p n -> p c n"))
                if first:
                    # consts slot in behind the first big load
                    nc.sync.dma_start(iw_t[:], iw_d[:])
                    nc.sync.dma_start(iota_t[:], iota_d[:])
                    first = False
                x2 = x2_pool.tile([128, 4, NROW], F16, tag="x2")
                for j in range(tch):
                    c = c0 + j
                    # per-chunk squares so chunk c's matmuls start as soon
                    # as its own square lands
                    if c in ACTE_CHUNKS:
                        nc.scalar.activation(
                            x2[:, j], xin[:, j],
                            mybir.ActivationFunctionType.Square)
                    else:
                        nc.vector.tensor_tensor(x2[:, j], xin[:, j],
                                                xin[:, j],
                                                op=mybir.AluOpType.mult)
                    oh = oh_pool.tile([128, RPAD], F16, tag="oh")
                    nc.vector.tensor_scalar(
                        oh[:], iota_t[:],
                        scalar1=iw_t[:, c:c + 1],
                        scalar2=iw_t[:, NCH + c:NCH + c + 1],
                        op0=mybir.AluOpType.is_equal,
                        op1=mybir.AluOpType.mult)
                    for g in range(NGRP):
                        nc.tensor.matmul(accs[g][:],
                                         lhsT=x2[:, j, g * 128:(g + 1) * 128],
                                         rhs=oh[:],
                                         start=(c == 0), stop=(c == NCH - 1))
                c0 += tch

            # psum -> sbuf: groups 0-3 on ScalarE, 4-7 on DVE (parallel);
            # each half gets its own output DMA
            res = const_pool.tile([128, NGRP * MAX_R], F32)
            h = 4 * MAX_R
            for g in range(4):
                nc.scalar.copy(res[:, g * MAX_R:(g + 1) * MAX_R],
                               accs[g][:, :MAX_R])
            for g in range(4, NGRP):
                nc.vector.tensor_copy(res[:, g * MAX_R:(g + 1) * MAX_R],
                                      accs[g][:, :MAX_R])
            nc.sync.dma_start(out_d[:, :h], res[:, :h])
            nc.sync.dma_start(out_d[:, h:], res[:, h:])

    nc.compile()
    return nc


def _get_program():
    if "nc" not in _CACHE:
        _CACHE["nc"] = _build_program()
    return _CACHE["nc"]


def _host_prep(shell_index: np.ndarray, shells_weight: np.ndarray,
               shells_count: np.ndarray):
    idx_flat = shell_index.reshape(-1).astype(np.int64)
    wt = (shells_weight.reshape(-1).astype(np.float64) / (
        shells_count.astype(np.float64)[idx_flat] + EPS)).astype(np.float32)
    # per-core packed [idx | wt], chunk-transposed: A[i, c] = v[c*128 + i]
    iw = []
    for k in range(NCORES):
        sl = slice(k * CPIX, (k + 1) * CPIX)
        iw.append(np.concatenate(
            [idx_flat[sl].reshape(NCH, 128).T.astype(np.float32),
             wt[sl].reshape(NCH, 128).T], axis=1).astype(np.float32))
    iota = np.broadcast_to(np.arange(RPAD, dtype=np.float16),
                           (128, RPAD)).copy()
    return iw, iota, idx_flat, wt


def kernel(x: np.ndarray, shell_index: np.ndarray,
           shells_weight: np.ndarray, shells_count: np.ndarray,
           _trace: bool = False, **_tr_kwargs) -> np.ndarray:
    assert x.shape == (B, C, S, XDIM)
    nc = _get_program()
    iw, iota, idx_flat, wt = _host_prep(shell_index, shells_weight,
                                        shells_count)

    xr = x.reshape(NROW, NPIX)
    x16 = (xr[:, :NPIX_DEV] * np.float32(PRESCALE)).astype(np.float16)
    in_maps = []
    for k in range(NCORES):
        xk = np.ascontiguousarray(x16[:, k * CPIX:(k + 1) * CPIX].T)
        in_maps.append({"xt": xk.reshape(NCH, 128, NROW), "iw": iw[k],
                        "iota": iota})

    # exact fp32 host path for the 256 residual pixels
    onehot = np.zeros((NPIX - NPIX_DEV, MAX_R), np.float32)
    onehot[np.arange(NPIX - NPIX_DEV), idx_flat[NPIX_DEV:]] = wt[NPIX_DEV:]
    xres = xr[:, NPIX_DEV:]
    host_part = (xres * xres) @ onehot                   # [1024, 129]

    res = run_bass_kernel_spmd(nc, in_maps, list(range(NCORES)),
                               trace=_trace, **_tr_kwargs)
    # each core returns [128, 8*129] f32 partial (1024x scaled) shell sums
    parts = np.stack([res.results[k]["out"] for k in range(NCORES)])
    full = parts.astype(np.float64).sum(axis=0) / (PRESCALE * PRESCALE)
    full = full.reshape(128, NGRP, MAX_R)
    # row-group g holds global rows g*128..(g+1)*128-1
    full = np.ascontiguousarray(full.transpose(1, 0, 2)).reshape(
        NROW, MAX_R).astype(np.float32)
    full = full + host_part
    full = full.reshape(B, C, MAX_R)
    if _trace:
        return full, res
    return full


# revision 16
# speedup vs baseline: 1.0124x; 1.0124x over previous
"""Radial power-spectrum (GroupStat.get_spectrum) Trainium2 kernel.

Math:  out[b,c,r] = sum_{p: idx[p]==r} x[b,c,p]^2 * w[p] / (cnt[r]+eps)

Strategy (8 NeuronCores, sharded over PIXELS, not batch):
  * All B*C = 1024 (b,c) rows on every core; each core owns 4096 of the
    33024 pixels (cores cover 32768; the 256-pixel remainder is summed
    exactly on the host in fp32 and added to the gathered result).
  * Host prep: transpose x to pixel-major [NPIX, 1024], scale by 32 and
    cast to fp16.  With 1024 rows per pixel the DMA lines are 2 KB, so
    the load runs at full HBM bandwidth AND lands with pixel on the
    partition dim -- no on-device transpose at all.
  * Device pipeline per 128-pixel chunk (32 per core):
      - DMA fp16 tiles [128p, 4, 1024n] (4 chunks per DMA; the last two
        tiles are 2 chunks so the post-DMA tail is short)
      - square in fp16 (values are 1024*x^2; the 32x host prescale keeps
        tiny x^2 out of fp16 subnormals), per-chunk ops balanced between
        ScalarE (14 chunks @1038ns) and DVE (18 chunks @594ns)
      - DVE: weighted one-hot [128p, 130r] = (iota == idx[p]) * wt[p],
        built ONCE per chunk and reused by all 8 row-groups
      - PE: for each of 8 row-groups g: psum_g[128n,130r] += x2T_g @ oh
  * psum_g -> SBUF f32 (4 copies on ScalarE, 4 on DVE, concurrently),
    two output DMAs overlap the copies; host sums the 8 per-core
    partials (pixel sharding => partial shell sums) and divides by 1024.
"""

import numpy as np

from concourse import bass, bacc, mybir
import concourse.tile as tile
from concourse.bass_utils import run_bass_kernel_spmd

B, C, S, XDIM = 128, 8, 256, 129
MAX_R = XDIM                # 129 shells
EPS = 1e-5
NCORES = 8
NROW = B * C                # 1024 total (b,c) rows
NGRP = NROW // 128          # 8 row-groups of 128
NPIX = S * XDIM             # 33024 pixels
NCH = 32                    # chunks of 128 pixels per core
CPIX = NCH * 128            # 4096 pixels per core
NPIX_DEV = NCORES * CPIX    # 32768 on device; 256 residual on host
RPAD = 130                  # even free dim for DVE 4x mode; col 129 unused
TILES = [4] * 7 + [2, 1, 1]  # chunks per DMA tile (sum = 32); tapered tail
PRESCALE = 32.0             # host multiplies x by 32 -> squares are 1024*x^2
# chunks whose square runs on ScalarE (~14/32; DVE also builds one-hots)
ACTE_CHUNKS = {0, 2, 4, 6, 9, 11, 13, 16, 18, 20, 22, 25, 27, 30}

F32 = mybir.dt.float32
F16 = mybir.dt.float16

_CACHE: dict = {}


def _build_program():
    nc = bacc.Bacc("TRN2", target_bir_lowering=False, debug=False,
                   num_devices=NCORES)

    # x, pre-transposed+scaled+fp16 on host: [chunk, pixel-in-chunk, row]
    x_d = nc.dram_tensor("xt", [NCH, 128, NROW], F16,
                         kind="ExternalInput").ap()
    # idx and wt packed: col c = idx for chunk c, col NCH+c = wt for chunk c
    iw_d = nc.dram_tensor("iw", [128, 2 * NCH], F32,
                          kind="ExternalInput").ap()
    iota_d = nc.dram_tensor("iota", [128, RPAD], F16,
                            kind="ExternalInput").ap()
    out_d = nc.dram_tensor("out", [128, NGRP * MAX_R], F32,
                           kind="ExternalOutput").ap()

    with tile.TileContext(nc) as tc:
        with tc.tile_pool(name="const", bufs=1) as const_pool, \
             tc.tile_pool(name="xin", bufs=4) as xin_pool, \
             tc.tile_pool(name="x2", bufs=4) as x2_pool, \
             tc.tile_pool(name="oh", bufs=8) as oh_pool, \
             tc.tile_pool(name="acc", bufs=1, space="PSUM") as acc_pool:

            accs = [acc_pool.tile([128, RPAD], F32, name=f"acc{g}")
                    for g in range(NGRP)]
            iw_t = const_pool.tile([128, 2 * NCH], F32)
            iota_t = const_pool.tile([128, RPAD], F16)

            c0 = 0
            first = True
            for tch in TILES:
                xin = xin_pool.tile([128, 4, NROW], F16, tag="xin")
                nc.sync.dma_start(
                    xin[:, :tch], x_d[c0:c0 + tch].rearrange("c p n -> p c n"))
                if first:
                    # consts slot in behind the first big load
                    nc.sync.dma_start(iw_t[:], iw_d[:])
                    nc.sync.dma_start(iota_t[:], iota_d[:])
                    first = False
                x2 = x2_pool.tile([128, 4, NROW], F16, tag="x2")
                for j in range(tch):
                    c = c0 + j
                    # per-chunk squares so chunk c's matmuls start as soon
                    # as its own square lands
                    if c in ACTE_CHUNKS:
                        nc.scalar.activation(
                            x2[:, j], xin[:, j],
                            mybir.ActivationFunctionType.Square)
                    else:
                        nc.vector.tensor_tensor(x2[:, j], xin[:, j],
                                                xin[:, j],
                                                op=mybir.AluOpType.mult)
                    oh = oh_pool.tile([128, RPAD], F16, tag="oh")
                    nc.vector.tensor_scalar(
                        oh[:], iota_t[:],
                        scalar1=iw_t[:, c:c + 1],
                        scalar2=iw_t[:, NCH + c:NCH + c + 1],
                        op0=mybir.AluOpType.is_equal,
                        op1=mybir.AluOpType.mult)
                    for g in range(NGRP):
                        nc.tensor.matmul(accs[g][:],
                                         lhsT=x2[:, j, g * 128:(g + 1) * 128],
                                         rhs=oh[:],
                                         start=(c == 0), stop=(c == NCH - 1))
                c0 += tch

            # psum -> sbuf: groups 0-3 on ScalarE, 4-7 on DVE (parallel);
            # each half gets its own output DMA
            res = const_pool.tile([128, NGRP * MAX_R], F32)
            h = 4 * MAX_R
            for g in range(4):
                nc.scalar.copy(res[:, g * MAX_R:(g + 1) * MAX_R],
                               accs[g][:, :MAX_R])
            for g in range(4, NGRP):
                nc.vector.tensor_copy(res[:, g * MAX_R:(g + 1) * MAX_R],
                                      accs[g][:, :MAX_R])
            nc.sync.dma_start(out_d[:, :h], res[:, :h])
            nc.sync.dma_start(out_d[:, h:], res[:, h:])

    nc.compile()
    return nc


def _get_program():
    if "nc" not in _CACHE:
        _CACHE["nc"] = _build_program()
    return _CACHE["nc"]


def _host_prep(shell_index: np.ndarray, shells_weight: np.ndarray,
               shells_count: np.ndarray):
    idx_flat = shell_index.reshape(-1).astype(np.int64)
    wt = (shells_weight.reshape(-1).astype(np.float64) / (
        shells_count.astype(np.float64)[idx_flat] + EPS)).astype(np.float32)
    # per-core packed [idx | wt], chunk-transposed: A[i, c] = v[c*128 + i]
    iw = []
    for k in range(NCORES):
        sl = slice(k * CPIX, (k + 1) * CPIX)
        iw.append(np.concatenate(
            [idx_flat[sl].reshape(NCH, 128).T.astype(np.float32),
             wt[sl].reshape(NCH, 128).T], axis=1).astype(np.float32))
    iota = np.broadcast_to(np.arange(RPAD, dtype=np.float16),
                           (128, RPAD)).copy()
    return iw, iota, idx_flat, wt


def kernel(x: np.ndarray, shell_index: np.ndarray,
           shells_weight: np.ndarray, shells_count: np.ndarray,
           _trace: bool = False, **_tr_kwargs) -> np.ndarray:
    assert x.shape == (B, C, S, XDIM)
    nc = _get_program()
    iw, iota, idx_flat, wt = _host_prep(shell_index, shells_weight,
                                        shells_count)

    xr = x.reshape(NROW, NPIX)
    x16 = (xr[:, :NPIX_DEV] * np.float32(PRESCALE)).astype(np.float16)
    in_maps = []
    for k in range(NCORES):
        xk = np.ascontiguousarray(x16[:, k * CPIX:(k + 1) * CPIX].T)
        in_maps.append({"xt": xk.reshape(NCH, 128, NROW), "iw": iw[k],
                        "iota": iota})

    # exact fp32 host path for the 256 residual pixels
    onehot = np.zeros((NPIX - NPIX_DEV, MAX_R), np.float32)
    onehot[np.arange(NPIX - NPIX_DEV), idx_flat[NPIX_DEV:]] = wt[NPIX_DEV:]
    xres = xr[:, NPIX_DEV:]
    host_part = (xres * xres) @ onehot                   # [1024, 129]

    res = run_bass_kernel_spmd(nc, in_maps, list(range(NCORES)),
                               trace=_trace, **_tr_kwargs)
    # each core returns [128, 8*129] f32 partial (1024x scaled) shell sums
    parts = np.stack([res.results[k]["out"] for k in range(NCORES)])
    full = parts.astype(np.float64).sum(axis=0) / (PRESCALE * PRESCALE)
    full = full.reshape(128, NGRP, MAX_R)
    # row-group g holds global rows g*128..(g+1)*128-1
    full = np.ascontiguousarray(full.transpose(1, 0, 2)).reshape(
        NROW, MAX_R).astype(np.float32)
    full = full + host_part
    full = full.reshape(B, C, MAX_R)
    if _trace:
        return full, res
    return full


# revision 17
# speedup vs baseline: 1.0261x; 1.0135x over previous
"""Radial power-spectrum (GroupStat.get_spectrum) Trainium2 kernel.

Math:  out[b,c,r] = sum_{p: idx[p]==r} x[b,c,p]^2 * w[p] / (cnt[r]+eps)

Strategy (8 NeuronCores, sharded over PIXELS, not batch):
  * All B*C = 1024 (b,c) rows on every core; each core owns 4096 of the
    33024 pixels (cores cover 32768; the 256-pixel remainder is summed
    exactly on the host in fp32 and added to the gathered result).
  * Host prep: transpose x to pixel-major [NPIX, 1024], scale by 32 and
    cast to fp16.  With 1024 rows per pixel the DMA lines are 2 KB, so
    the load runs at full HBM bandwidth AND lands with pixel on the
    partition dim -- no on-device transpose at all.
  * Device pipeline per 128-pixel chunk (32 per core):
      - DMA fp16 tiles [128p, 4, 1024n] (4 chunks per DMA; the last two
        tiles are 2 chunks so the post-DMA tail is short)
      - square in fp16 (values are 1024*x^2; the 32x host prescale keeps
        tiny x^2 out of fp16 subnormals), per-chunk ops balanced between
        ScalarE (14 chunks @1038ns) and DVE (18 chunks @594ns)
      - DVE: weighted one-hot [128p, 130r] = (iota == idx[p]) * wt[p],
        built ONCE per chunk and reused by all 8 row-groups
      - PE: for each of 8 row-groups g: psum_g[128n,130r] += x2T_g @ oh
  * psum_g -> SBUF f32 (4 copies on ScalarE, 4 on DVE, concurrently),
    two output DMAs overlap the copies; host sums the 8 per-core
    partials (pixel sharding => partial shell sums) and divides by 1024.
"""

import numpy as np

from concourse import bass, bacc, mybir
import concourse.tile as tile
from concourse.bass_utils import run_bass_kernel_spmd

B, C, S, XDIM = 128, 8, 256, 129
MAX_R = XDIM                # 129 shells
EPS = 1e-5
NCORES = 8
NROW = B * C                # 1024 total (b,c) rows
NGRP = NROW // 128          # 8 row-groups of 128
NPIX = S * XDIM             # 33024 pixels
NCH = 32                    # chunks of 128 pixels per core
CPIX = NCH * 128            # 4096 pixels per core
NPIX_DEV = NCORES * CPIX    # 32768 on device; 256 residual on host
RPAD = 130                  # even free dim for DVE 4x mode; col 129 unused
TILES = [4] * 7 + [2, 1, 1]  # chunks per DMA tile (sum = 32); tapered tail
PRESCALE = 32.0             # host multiplies x by 32 -> squares are 1024*x^2
# chunks whose square runs on ScalarE (~14/32; DVE also builds one-hots)
ACTE_CHUNKS = {0, 2, 4, 6, 9, 11, 13, 16, 18, 20, 22, 25, 27, 30}

F32 = mybir.dt.float32
F16 = mybir.dt.float16

_CACHE: dict = {}


def _build_program():
    nc = bacc.Bacc("TRN2", target_bir_lowering=False, debug=False,
                   num_devices=NCORES)

    # x, pre-transposed+scaled+fp16 on host: [chunk, pixel-in-chunk, row]
    x_d = nc.dram_tensor("xt", [NCH, 128, NROW], F16,
                         kind="ExternalInput").ap()
    # idx and wt packed: col c = idx for chunk c, col NCH+c = wt for chunk c
    iw_d = nc.dram_tensor("iw", [128, 2 * NCH], F32,
                          kind="ExternalInput").ap()
    iota_d = nc.dram_tensor("iota", [128, RPAD], F16,
                            kind="ExternalInput").ap()
    out_d = nc.dram_tensor("out", [128, NGRP * MAX_R], F16,
                           kind="ExternalOutput").ap()

    with tile.TileContext(nc) as tc:
        with tc.tile_pool(name="const", bufs=1) as const_pool, \
             tc.tile_pool(name="xin", bufs=4) as xin_pool, \
             tc.tile_pool(name="x2", bufs=4) as x2_pool, \
             tc.tile_pool(name="oh", bufs=8) as oh_pool, \
             tc.tile_pool(name="acc", bufs=1, space="PSUM") as acc_pool:

            accs = [acc_pool.tile([128, RPAD], F32, name=f"acc{g}")
                    for g in range(NGRP)]
            iw_t = const_pool.tile([128, 2 * NCH], F32)
            iota_t = const_pool.tile([128, RPAD], F16)

            c0 = 0
            first = True
            for tch in TILES:
                xin = xin_pool.tile([128, 4, NROW], F16, tag="xin")
                nc.sync.dma_start(
                    xin[:, :tch], x_d[c0:c0 + tch].rearrange("c p n -> p c n"))
                if first:
                    # consts slot in behind the first big load
                    nc.sync.dma_start(iw_t[:], iw_d[:])
                    nc.sync.dma_start(iota_t[:], iota_d[:])
                    first = False
                x2 = x2_pool.tile([128, 4, NROW], F16, tag="x2")
                for j in range(tch):
                    c = c0 + j
                    # per-chunk squares so chunk c's matmuls start as soon
                    # as its own square lands
                    if c in ACTE_CHUNKS:
                        nc.scalar.activation(
                            x2[:, j], xin[:, j],
                            mybir.ActivationFunctionType.Square)
                    else:
                        nc.vector.tensor_tensor(x2[:, j], xin[:, j],
                                                xin[:, j],
                                                op=mybir.AluOpType.mult)
                    oh = oh_pool.tile([128, RPAD], F16, tag="oh")
                    nc.vector.tensor_scalar(
                        oh[:], iota_t[:],
                        scalar1=iw_t[:, c:c + 1],
                        scalar2=iw_t[:, NCH + c:NCH + c + 1],
                        op0=mybir.AluOpType.is_equal,
                        op1=mybir.AluOpType.mult)
                    for g in range(NGRP):
                        nc.tensor.matmul(accs[g][:],
                                         lhsT=x2[:, j, g * 128:(g + 1) * 128],
                                         rhs=oh[:],
                                         start=(c == 0), stop=(c == NCH - 1))
                c0 += tch

            # psum -> sbuf: groups 0-3 on ScalarE, 4-7 on DVE (parallel);
            # each half gets its own output DMA
            res = const_pool.tile([128, NGRP * MAX_R], F16)
            h = 4 * MAX_R
            for g in range(4):
                nc.scalar.copy(res[:, g * MAX_R:(g + 1) * MAX_R],
                               accs[g][:, :MAX_R])
            for g in range(4, NGRP):
                nc.vector.tensor_copy(res[:, g * MAX_R:(g + 1) * MAX_R],
                                      accs[g][:, :MAX_R])
            nc.sync.dma_start(out_d[:, :h], res[:, :h])
            nc.sync.dma_start(out_d[:, h:], res[:, h:])

    nc.compile()
    return nc


def _get_program():
    if "nc" not in _CACHE:
        _CACHE["nc"] = _build_program()
    return _CACHE["nc"]


def _host_prep(shell_index: np.ndarray, shells_weight: np.ndarray,
               shells_count: np.ndarray):
    idx_flat = shell_index.reshape(-1).astype(np.int64)
    wt = (shells_weight.reshape(-1).astype(np.float64) / (
        shells_count.astype(np.float64)[idx_flat] + EPS)).astype(np.float32)
    # per-core packed [idx | wt], chunk-transposed: A[i, c] = v[c*128 + i]
    iw = []
    for k in range(NCORES):
        sl = slice(k * CPIX, (k + 1) * CPIX)
        iw.append(np.concatenate(
            [idx_flat[sl].reshape(NCH, 128).T.astype(np.float32),
             wt[sl].reshape(NCH, 128).T], axis=1).astype(np.float32))
    iota = np.broadcast_to(np.arange(RPAD, dtype=np.float16),
                           (128, RPAD)).copy()
    return iw, iota, idx_flat, wt


def kernel(x: np.ndarray, shell_index: np.ndarray,
           shells_weight: np.ndarray, shells_count: np.ndarray,
           _trace: bool = False, **_tr_kwargs) -> np.ndarray:
    assert x.shape == (B, C, S, XDIM)
    nc = _get_program()
    iw, iota, idx_flat, wt = _host_prep(shell_index, shells_weight,
                                        shells_count)

    xr = x.reshape(NROW, NPIX)
    x16 = (xr[:, :NPIX_DEV] * np.float32(PRESCALE)).astype(np.float16)
    in_maps = []
    for k in range(NCORES):
        xk = np.ascontiguousarray(x16[:, k * CPIX:(k + 1) * CPIX].T)
        in_maps.append({"xt": xk.reshape(NCH, 128, NROW), "iw": iw[k],
                        "iota": iota})

    # exact fp32 host path for the 256 residual pixels
    onehot = np.zeros((NPIX - NPIX_DEV, MAX_R), np.float32)
    onehot[np.arange(NPIX - NPIX_DEV), idx_flat[NPIX_DEV:]] = wt[NPIX_DEV:]
    xres = xr[:, NPIX_DEV:]
    host_part = (xres * xres) @ onehot                   # [1024, 129]

    res = run_bass_kernel_spmd(nc, in_maps, list(range(NCORES)),
                               trace=_trace, **_tr_kwargs)
    # each core returns [128, 8*129] f32 partial (1024x scaled) shell sums
    parts = np.stack([res.results[k]["out"] for k in range(NCORES)])
    full = parts.astype(np.float64).sum(axis=0) / (PRESCALE * PRESCALE)
    full = full.reshape(128, NGRP, MAX_R)
    # row-group g holds global rows g*128..(g+1)*128-1
    full = np.ascontiguousarray(full.transpose(1, 0, 2)).reshape(
        NROW, MAX_R).astype(np.float32)
    full = full + host_part
    full = full.reshape(B, C, MAX_R)
    if _trace:
        return full, res
    return full


# revision 18
# speedup vs baseline: 1.0778x; 1.0505x over previous
"""Radial power-spectrum (GroupStat.get_spectrum) Trainium2 kernel.

Math:  out[b,c,r] = sum_{p: idx[p]==r} x[b,c,p]^2 * w[p] / (cnt[r]+eps)

Strategy (8 NeuronCores, sharded over PIXELS sorted by shell):
  * All B*C = 1024 (b,c) rows on every core.  Pixels are sorted by shell
    index on the host; each core owns 4096 consecutive sorted pixels
    (cores cover 32768; the 256-pixel remainder is summed exactly on the
    host in fp32).  Sorted pixels mean each core's shells span a narrow
    band (<=52), so the one-hot / matmul / psum / output are only
    RBAND=56 wide instead of 129.
  * Host prep: gather+transpose x to pixel-major [4096, 1024] per core,
    scale by 32, cast fp16.  With 1024 rows per pixel the DMA lines are
    2 KB, so the load runs at full HBM bandwidth AND lands with pixel on
    the partition dim -- no on-device transpose at all.
  * Device pipeline per 128-pixel chunk (32 per core):
      - DMA fp16 tiles [128p, 4, 1024n] (4 chunks per DMA; tapered tail)
      - square in fp16 (values are 1024*x^2; the 32x host prescale keeps
        tiny x^2 out of fp16 subnormals), per-chunk ops balanced between
        ScalarE (~1038ns) and DVE (~594ns)
      - DVE: weighted one-hot [128p, 56r] = (iota == local_idx[p]) * wt[p],
        built ONCE per chunk and reused by all 8 row-groups
      - PE: for each of 8 row-groups g: psum_g[128n,56r] += x2T_g @ oh
  * psum_g -> SBUF fp16 (4 copies on ScalarE, 4 on DVE, concurrently),
    one small output DMA [128, 8*56] fp16; host scatter-adds each core's
    shell band into the full result and divides by 1024.
"""

import numpy as np

from concourse import bass, bacc, mybir
import concourse.tile as tile
from concourse.bass_utils import run_bass_kernel_spmd

B, C, S, XDIM = 128, 8, 256, 129
MAX_R = XDIM                # 129 shells
EPS = 1e-5
NCORES = 8
NROW = B * C                # 1024 total (b,c) rows
NGRP = NROW // 128          # 8 row-groups of 128
NPIX = S * XDIM             # 33024 pixels
NCH = 32                    # chunks of 128 pixels per core
CPIX = NCH * 128            # 4096 pixels per core
NPIX_DEV = NCORES * CPIX    # 32768 on device; 256 residual on host
RBAND = 56                  # max shells per core's sorted band (pad, even)
TILES = [4] * 7 + [2, 1, 1]  # chunks per DMA tile (sum = 32); tapered tail
PRESCALE = 32.0             # host multiplies x by 32 -> squares are 1024*x^2
# chunks whose square runs on ScalarE (~14/32; DVE also builds one-hots)
ACTE_CHUNKS = {0, 2, 4, 6, 9, 11, 13, 16, 18, 20, 22, 25, 27, 30}

F32 = mybir.dt.float32
F16 = mybir.dt.float16

_CACHE: dict = {}


def _build_program():
    nc = bacc.Bacc("TRN2", target_bir_lowering=False, debug=False,
                   num_devices=NCORES)

    # x, sorted+gathered+scaled+fp16 on host: [chunk, pixel-in-chunk, row]
    x_d = nc.dram_tensor("xt", [NCH, 128, NROW], F16,
                         kind="ExternalInput").ap()
    # local idx and wt packed: col c = idx chunk c, col NCH+c = wt chunk c
    iw_d = nc.dram_tensor("iw", [128, 2 * NCH], F32,
                          kind="ExternalInput").ap()
    iota_d = nc.dram_tensor("iota", [128, RBAND], F16,
                            kind="ExternalInput").ap()
    out_d = nc.dram_tensor("out", [128, NGRP * RBAND], F16,
                           kind="ExternalOutput").ap()

    with tile.TileContext(nc) as tc:
        with tc.tile_pool(name="const", bufs=1) as const_pool, \
             tc.tile_pool(name="xin", bufs=4) as xin_pool, \
             tc.tile_pool(name="x2", bufs=4) as x2_pool, \
             tc.tile_pool(name="oh", bufs=8) as oh_pool, \
             tc.tile_pool(name="acc", bufs=1, space="PSUM") as acc_pool:

            accs = [acc_pool.tile([128, RBAND], F32, name=f"acc{g}")
                    for g in range(NGRP)]
            iw_t = const_pool.tile([128, 2 * NCH], F32)
            iota_t = const_pool.tile([128, RBAND], F16)

            c0 = 0
            first = True
            for tch in TILES:
                xin = xin_pool.tile([128, 4, NROW], F16, tag="xin")
                nc.sync.dma_start(
                    xin[:, :tch], x_d[c0:c0 + tch].rearrange("c p n -> p c n"))
                if first:
                    # consts slot in behind the first big load
                    nc.sync.dma_start(iw_t[:], iw_d[:])
                    nc.sync.dma_start(iota_t[:], iota_d[:])
                    first = False
                x2 = x2_pool.tile([128, 4, NROW], F16, tag="x2")
                for j in range(tch):
                    c = c0 + j
                    # per-chunk squares so chunk c's matmuls start as soon
                    # as its own square lands
                    if c in ACTE_CHUNKS:
                        nc.scalar.activation(
                            x2[:, j], xin[:, j],
                            mybir.ActivationFunctionType.Square)
                    else:
                        nc.vector.tensor_tensor(x2[:, j], xin[:, j],
                                                xin[:, j],
                                                op=mybir.AluOpType.mult)
                    oh = oh_pool.tile([128, RBAND], F16, tag="oh")
                    nc.vector.tensor_scalar(
                        oh[:], iota_t[:],
                        scalar1=iw_t[:, c:c + 1],
                        scalar2=iw_t[:, NCH + c:NCH + c + 1],
                        op0=mybir.AluOpType.is_equal,
                        op1=mybir.AluOpType.mult)
                    for g in range(NGRP):
                        nc.tensor.matmul(accs[g][:],
                                         lhsT=x2[:, j, g * 128:(g + 1) * 128],
                                         rhs=oh[:],
                                         start=(c == 0), stop=(c == NCH - 1))
                c0 += tch

            # psum -> sbuf: groups 0-3 on ScalarE, 4-7 on DVE (parallel)
            res = const_pool.tile([128, NGRP * RBAND], F16)
            for g in range(4):
                nc.scalar.copy(res[:, g * RBAND:(g + 1) * RBAND], accs[g][:])
            for g in range(4, NGRP):
                nc.vector.tensor_copy(res[:, g * RBAND:(g + 1) * RBAND],
                                      accs[g][:])
            nc.sync.dma_start(out_d[:], res[:])

    nc.compile()
    return nc


def _get_program():
    if "nc" not in _CACHE:
        _CACHE["nc"] = _build_program()
    return _CACHE["nc"]


def kernel(x: np.ndarray, shell_index: np.ndarray,
           shells_weight: np.ndarray, shells_count: np.ndarray,
           _trace: bool = False, **_tr_kwargs) -> np.ndarray:
    assert x.shape == (B, C, S, XDIM)
    nc = _get_program()

    idx_flat = shell_index.reshape(-1).astype(np.int64)
    wt = (shells_weight.reshape(-1).astype(np.float64) / (
        shells_count.astype(np.float64)[idx_flat] + EPS)).astype(np.float32)
    order = np.argsort(idx_flat, kind="stable")

    xr = x.reshape(NROW, NPIX)
    x16 = (xr * np.float32(PRESCALE)).astype(np.float16)

    in_maps = []
    r_lo = []
    iota = np.broadcast_to(np.arange(RBAND, dtype=np.float16),
                           (128, RBAND)).copy()
    for k in range(NCORES):
        pix = order[k * CPIX:(k + 1) * CPIX]
        idx_k = idx_flat[pix]
        lo = int(idx_k[0])               # sorted: min is first
        assert int(idx_k[-1]) - lo < RBAND, (k, lo, int(idx_k[-1]))
        r_lo.append(lo)
        xk = np.ascontiguousarray(x16[:, pix].T)
        iw_k = np.concatenate(
            [(idx_k - lo).reshape(NCH, 128).T.astype(np.float32),
             wt[pix].reshape(NCH, 128).T], axis=1).astype(np.float32)
        in_maps.append({"xt": xk.reshape(NCH, 128, NROW), "iw": iw_k,
                        "iota": iota})

    # exact fp32 host path for the 256 residual (highest-shell) pixels
    pix_res = order[NPIX_DEV:]
    onehot = np.zeros((NPIX - NPIX_DEV, MAX_R), np.float32)
    onehot[np.arange(NPIX - NPIX_DEV), idx_flat[pix_res]] = wt[pix_res]
    xres = xr[:, pix_res]
    host_part = (xres * xres) @ onehot                   # [1024, 129]

    res = run_bass_kernel_spmd(nc, in_maps, list(range(NCORES)),
                               trace=_trace, **_tr_kwargs)
    # each core returns [128, 8*56] fp16 partials (1024x scaled) for its band
    full = np.zeros((NROW, MAX_R), np.float64)
    for k in range(NCORES):
        part = np.asarray(res.results[k]["out"], dtype=np.float64)
        part = part.reshape(128, NGRP, RBAND).transpose(1, 0, 2).reshape(
            NROW, RBAND)                                  # [1024, 56]
        w = min(RBAND, MAX_R - r_lo[k])
        full[:, r_lo[k]:r_lo[k] + w] += part[:, :w]
    full = (full / (PRESCALE * PRESCALE)).astype(np.float32) + host_part
    full = full.reshape(B, C, MAX_R)
    if _trace:
        return full, res
    return full
